# revision 42
# baseline (speedup 1.0000x reference)
"""Multi-head causal self-attention (q=k=v bug faithful) on 8 trn2 cores.

Sharding: 24 (batch, head) jobs -> 3 heads per core. Core c: batch c//4,
heads (c%4)*3 .. +3. Each core computes its heads' attention outputs and a
partial output-projection Z^T = sum_h O_h @ Wout_slice_h  (shape [768, 4096],
bf16). Host: sum the 4 partials per batch (bias folded into the matmul via a
ones row on one core per batch), transpose to [4096, 768].

Device algorithm per core (bf16 matmuls, fp32 PSUM accumulation):
  1. Q^T[h] duplicated into 128 partitions (rows 0-63 and 64-127 identical)
     via duplicated Wq columns -- enables 2x row-tiled score matmuls.
     Q natural (qn) is produced by a second matmul against Wq in natural
     layout (lhsT = the same X^T tiles), replacing the serial DMA-xbar
     transpose phase of the previous version.
  2. flash-style, i-groups of 512, j-blocks of 128 (causal-skipped):
       S^T[jb, i] = Q^T[:,jb].T @ Q^T[:,i-group]  row-tiled: even jb on PE
         rows 0-63, odd jb on rows 64-127, concurrent (tile_position)
       P^T = exp(S^T): split between ScalarE (table exp) and DVE using the
         Schraudolph bit trick out = bitcast_bf16(u16(184.665*x + 16250.4)),
         one tensor_scalar pass. i-group 0 (rows with few softmax terms,
         where the +-3% sawtooth does not average out) is ScalarE-only.
       diag band masked by upper-tri 0/1 mask multiply (DVE, bf16)
       [O | denom]^T += [Q[jb]|1].T @ P^T                   (PSUM accum)
     normalize: O^T *= 1/denom (DVE reciprocal_approx_fast + gpsimd
     partition_broadcast + DVE multiply)
  3. Z^T[oc, i] = [Wout_slice; bias].T @ [O_cat^T; ones]  (bias fused as a
     ones row in ot2 / 65th row of wout1), staged to SBUF bf16, DMA out.
"""

import os

import numpy as np

B, L, D, H, HS = 2, 4096, 768, 12, 64
NCORES = 8
HPC = 3  # heads per core
IG = 512  # i-group width
NIG = L // IG
SCALE = 1.0 / np.sqrt(np.float32(D))
SQS = np.sqrt(SCALE).astype(np.float32)  # folded into Wq (and undone in Wout)

# Schraudolph exp approximation constants (bf16 bit pattern via uint16):
#   exp(x) ~= bitcast_bf16(uint16(round(A16*x + B16)))
A16 = float(128.0 / np.log(2.0))
B16 = float(16256.0 - 0.0435 * 128.0)

_cached = {}


def _build_program():
    import concourse.bass as bass
    import concourse.mybir as mybir
    import concourse.tile as tile
    from concourse import bacc
    from concourse.masks import make_upper_triangular

    f32 = mybir.dt.float32
    bf16 = mybir.dt.bfloat16
    u16 = mybir.dt.uint16
    Exp = mybir.ActivationFunctionType.Exp
    Copy = mybir.ActivationFunctionType.Copy
    MUL = mybir.AluOpType.mult
    ADD = mybir.AluOpType.add

    nc = bacc.Bacc(
        "TRN2",
        target_bir_lowering=False,
        debug=False,
        enable_asserts=False,
        num_devices=NCORES,
    )

    xT = nc.dram_tensor("xT", [D, L], bf16, kind="ExternalInput").ap()
    # duplicated per-head Wq^T: cols h*128..h*128+64 == cols +64..+128
    wqd = nc.dram_tensor("wqd", [D, HPC * 128], bf16, kind="ExternalInput").ap()
    # wout rows 0..191 = W_out slice ^T / SQS, row 192 = bias (or 0)
    wout = nc.dram_tensor("wout", [HPC * HS + 1, D], bf16, kind="ExternalInput").ap()
    zT = nc.dram_tensor("zT", [D, L], bf16, kind="ExternalOutput").ap()

    xT_r = xT.rearrange("(c p) i -> p c i", p=128)  # [128, 6, L]
    zT_r = zT.rearrange("(c p) i -> c p i", p=128)  # [6, 128, L]

    with tile.TileContext(nc) as tc:
        with (
            tc.tile_pool(name="consts", bufs=1) as consts,
            tc.tile_pool(name="persist", bufs=1) as persist,
        ):
            # ---- constants ----
            wqd_sb = consts.tile([128, 6, HPC * 128], bf16)
            nc.sync.dma_start(out=wqd_sb, in_=wqd.rearrange("(c p) m -> p c m", p=128))
            wout0_sb = consts.tile([128, D], bf16)
            nc.sync.dma_start(out=wout0_sb, in_=wout[0:128, :])
            wout1_sb = consts.tile([65, D], bf16)
            nc.sync.dma_start(out=wout1_sb, in_=wout[128:193, :])
            # keep mask[p, t] = 1.0 where t >= p else 0.0
            trimask = consts.tile([128, 128], bf16)
            make_upper_triangular(nc, trimask, val=1.0, diag=True)

            # ---- persistent per-head state ----
            # Q^T per head duplicated: rows 0-63 = rows 64-127 = Q^T
            qts = [persist.tile([128, L], bf16, name=f"qt{h}") for h in range(HPC)]
            # Q natural + ones column: [128, block, head, 80]; col 64 = 1.0
            qn = persist.tile([128, 32, HPC, 80], bf16)
            nc.vector.memset(qn[:, :, :, 64:65], 1.0)
            # O^T in per-i-group tiles so the output projection's reads only
            # depend on that i-group's normalize, not the whole tensor
            ot01g = [
                persist.tile([128, IG], bf16, name=f"ot01_{g}") for g in range(NIG)
            ]
            ot2g = [
                persist.tile([65, IG], bf16, name=f"ot2_{g}") for g in range(NIG)
            ]
            for g in range(NIG):
                nc.vector.memset(ot2g[g][64:65, :], 1.0)

            def ot_h(h, ig):
                if h < 2:
                    return ot01g[ig][h * 64 : (h + 1) * 64, :]
                return ot2g[ig][0:64, :]

            # ---- phase 1: Q^T (duplicated) and Q natural projections ----
            with (
                tc.tile_pool(name="xin", bufs=2) as xin,
                tc.tile_pool(name="qps", bufs=3, space="PSUM") as qps,
            ):
                for ig in range(NIG):
                    i0 = ig * IG
                    xt = xin.tile([128, 6, IG], bf16, tag="xt")
                    nc.sync.dma_start(out=xt, in_=xT_r[:, :, i0 : i0 + IG])
                    for h in range(HPC):
                        qp = qps.tile([128, IG], f32, tag="qp")
                        for c in range(6):
                            nc.tensor.matmul(
                                qp,
                                lhsT=wqd_sb[:, c, h * 128 : (h + 1) * 128],
                                rhs=xt[:, c, :],
                                start=(c == 0),
                                stop=(c == 5),
                            )
                        nc.scalar.activation(
                            out=qts[h][:, i0 : i0 + IG], in_=qp, func=Copy
                        )
                    # Q natural via DVE 32x32 stream-transposes of the
                    # (duplicated) Q^T rows 64-127, batched 8 blocks (2 igs)
                    # at a time so they hide under later phase-1 PE work
                    if ig % 2 == 1:
                        b0 = (ig - 1) * 4
                        for h in range(HPC):
                            for eb in range(2):
                                src_rows = qts[h][
                                    64 + 32 * eb : 96 + 32 * eb, :
                                ].rearrange("p (b s c) -> p b s c", s=4, c=32)
                                for ia in range(4):
                                    nc.vector.transpose(
                                        out=qn[
                                            32 * ia : 32 * ia + 32,
                                            b0 : b0 + 8,
                                            h,
                                            32 * eb : 32 * eb + 32,
                                        ],
                                        in_=src_rows[:, b0 : b0 + 8, ia, :],
                                    )

            # ---- phase 2: attention + output projection ----
            expctr = [0]
            with (
                tc.tile_pool(name="scps", bufs=3, space="PSUM") as scps,
                tc.tile_pool(name="avps", bufs=2, space="PSUM") as avps,
                tc.tile_pool(name="ptp", bufs=6) as ptp,
                tc.tile_pool(name="nrm", bufs=6) as nrm,
            ):
                # deepest sweep first: fills the pipeline right at the
                # phase boundary and leaves the shallow i-groups (which
                # cannot fill the pipeline) for the end
                for ig in reversed(range(NIG)):
                    i0 = ig * IG
                    jb_max = 4 * (ig + 1)
                    for h in range(HPC):
                        av = avps.tile([65, IG], f32, tag="av")
                        njg = jb_max // 2
                        scs, pts = {}, {}

                        def emit_scores(jg, h=h, ig=ig, i0=i0):
                            sc = scps.tile([128, 2, IG], f32, tag="sc", name="sc")
                            pt = ptp.tile([128, 2, IG], u16, tag="pt", name="pt")
                            scs[jg], pts[jg] = sc, pt
                            for k in range(2):
                                jb = jg * 2 + k
                                r = jb - 4 * ig
                                sr = 128 * r if r > 0 else 0
                                nc.tensor.matmul(
                                    sc[:, k, sr:],
                                    lhsT=qts[h][
                                        64 * k : 64 * k + 64,
                                        jb * 128 : (jb + 1) * 128,
                                    ],
                                    rhs=qts[h][64 * k : 64 * k + 64, i0 + sr : i0 + IG],
                                    start=True,
                                    stop=True,
                                )

                        # software pipeline: keep TWO jg of scores in flight
                        # ahead of the exp/av consumers, so the AV matmul at
                        # the PE queue head never waits on exp (~1.1us) with
                        # only one jg (~0.4us) of scores to hide it
                        emit_scores(0)
                        if njg > 1:
                            emit_scores(1)
                        for jg in range(njg):
                            sc, pt = scs.pop(jg), pts.pop(jg)
                            ptb = pt[:].bitcast(bf16)
                            diag = (jg * 2 + 1) - 4 * ig >= 0
                            # i-group 0 rows have few softmax terms; keep them
                            # on the exact ScalarE path (see module docstring)
                            if ig == 0 and jg == 0:
                                eng = "s"
                            elif ig == 7 and jg < 8:
                                # DVE is draining the last qn transpose batch
                                eng = "s"
                            else:
                                eng = "sdsdsdsdd"[expctr[0] % 9]
                                expctr[0] += 1
                            if diag:
                                spans = []
                                for k in range(2):
                                    r = jg * 2 + k - 4 * ig
                                    sr = 128 * r if r > 0 else 0
                                    spans.append((k, sr))
                            else:
                                spans = [(None, 0)]
                            for k, sr in spans:
                                src = sc[:, :, :] if k is None else sc[:, k, sr:]
                                if eng == "s":
                                    dst = ptb if k is None else ptb[:, k, sr:]
                                    nc.scalar.activation(out=dst, in_=src, func=Exp)
                                else:
                                    dst = pt[:, :, :] if k is None else pt[:, k, sr:]
                                    nc.vector.tensor_scalar(
                                        dst, src, A16, B16, MUL, ADD
                                    )
                            if jg + 2 < njg:
                                emit_scores(jg + 2)
                            for k in range(2):
                                jb = jg * 2 + k
                                r = jb - 4 * ig
                                sr = 128 * r if r > 0 else 0
                                if r >= 0:  # diagonal band: zero out j > i
                                    nc.vector.tensor_mul(
                                        ptb[:, k, sr : sr + 128],
                                        ptb[:, k, sr : sr + 128],
                                        trimask,
                                    )
                                nc.tensor.matmul(
                                    av[:, sr:IG],
                                    lhsT=qn[:, jb, h, 0:65],
                                    rhs=ptb[:, k, sr:IG],
                                    start=(jb == 0),
                                    stop=(jb == jb_max - 1),
                                    skip_group_check=True,
                                )
                        # custom-DVE ops misread PSUM: stage denom row to SBUF
                        dsb = nrm.tile([1, IG], f32, tag="dsb")
                        nc.scalar.activation(out=dsb, in_=av[64:65, :], func=Copy)
                        recip = nrm.tile([1, IG], f32, tag="recip")
                        nc.vector.reciprocal_approx_fast(recip, dsb)
                        rb = nrm.tile([64, IG], f32, tag="rb")
                        nc.gpsimd.partition_broadcast(rb, recip, channels=64)
                        nc.vector.tensor_mul(ot_h(h, ig), av[0:64, :], rb)

            # ---- phase 3: output projection (stationary weights reused
            # across all i-groups per output chunk) ----
            with (
                tc.tile_pool(name="ztps", bufs=4, space="PSUM") as ztps,
                tc.tile_pool(name="ztb", bufs=4) as ztb,
            ):
                for oc in range(6):
                    for ig in range(NIG):
                        i0 = ig * IG
                        zt = ztps.tile([128, IG], f32, tag="zt")
                        nc.tensor.matmul(
                            zt,
                            lhsT=wout0_sb[:, oc * 128 : (oc + 1) * 128],
                            rhs=ot01g[ig],
                            start=True,
                            stop=False,
                        )
                        nc.tensor.matmul(
                            zt,
                            lhsT=wout1_sb[:, oc * 128 : (oc + 1) * 128],
                            rhs=ot2g[ig],
                            start=False,
                            stop=True,
                        )
                        zb = ztb.tile([128, IG], bf16, tag="zb")
                        if ig % 2 == 0:
                            nc.scalar.activation(out=zb, in_=zt, func=Copy)
                        else:
                            nc.vector.tensor_copy(out=zb, in_=zt)
                        nc.sync.dma_start(out=zT_r[oc, :, i0 : i0 + IG], in_=zb)

    nc.compile()
    return nc


def _get_program():
    if "nc" not in _cached:
        _cached["nc"] = _build_program()
    return _cached["nc"]


def _make_in_maps(x, Wq, W_out, b_out):
    import ml_dtypes

    bf16 = ml_dtypes.bfloat16
    x = np.asarray(x, dtype=np.float32)
    Wq = np.asarray(Wq, dtype=np.float32)
    W_out = np.asarray(W_out, dtype=np.float32)
    b_out = np.asarray(b_out, dtype=np.float32)
    in_maps = []
    for c in range(NCORES):
        b = c // (NCORES // B)
        hg = c % (NCORES // B)
        h0 = hg * HPC
        xT = np.ascontiguousarray(x[b].T).astype(bf16)  # [D, L]
        wq = Wq[h0 : h0 + HPC] * SQS  # [3, 64, D]
        # natural layout [D, 3*64]
        wqn = np.ascontiguousarray(wq.transpose(2, 0, 1).reshape(D, HPC * HS))
        # duplicated layout [D, 3*128]
        wqd = np.empty((D, HPC * 128), dtype=np.float32)
        for h in range(HPC):
            wqd[:, h * 128 : h * 128 + 64] = wqn[:, h * 64 : (h + 1) * 64]
            wqd[:, h * 128 + 64 : (h + 1) * 128] = wqn[:, h * 64 : (h + 1) * 64]
        wout = np.empty((HPC * HS + 1, D), dtype=np.float32)
        wout[0 : HPC * HS] = W_out[:, h0 * HS : (h0 + HPC) * HS].T / SQS
        wout[HPC * HS] = b_out if hg == 0 else 0.0
        in_maps.append(
            {"xT": xT, "wqd": wqd.astype(bf16), "wout": wout.astype(bf16)}
        )
    return in_maps


def run(x, Wq, W_out, b_out, trace=False):
    from concourse.bass_utils import run_bass_kernel_spmd

    nc = _get_program()
    in_maps = _make_in_maps(x, Wq, W_out, b_out)
    res = run_bass_kernel_spmd(
        nc, in_maps, core_ids=list(range(NCORES)), trace=trace
    )
    partials = [r["zT"] for r in res.results]  # each [D, L] bf16
    out = np.empty((B, L, D), dtype=np.float32)
    for b in range(B):
        g = NCORES // B
        acc = partials[b * g].astype(np.float32)
        for c in range(b * g + 1, (b + 1) * g):
            acc += partials[c].astype(np.float32)
        out[b] = acc.T
    return out, res


def kernel(x, Wq, W_out, b_out):
    out, _ = run(
        x, Wq, W_out, b_out, trace=bool(int(os.environ.get("KERNEL_TRACE", "0")))
    )
    return out
